# revision 6
# baseline (speedup 1.0000x reference)
"""Trainium2 Bass kernel for RoPE linear attention (no softmax, strict causal).

Computes: QR = rope(Q); S = tril(QR @ QR^T, -1); out = S @ V
for Q [B=2, H=8, T=2048, N=1024], V [B,H,T,D=128], K == Q.

Sharding: B*H = 16 (b,h) pairs -> 2 per core across 8 cores (fully parallel).

Per-core device pipeline (per (b,h)):
  1. load Q tiles fp32 [128, 1024]
  2. cast fp32->fp16 with pair de-interleave (evens -> cols 0:512, odds -> 512:1024)
  3. DMA xbar transpose -> QRT fp16 [n', t]: 8 chunks [128, 2048]
     (chunks 0..3 = even pair-halves, 4..7 = odd halves)
  4. rope in transposed layout (unit-stride fp16 tensor_tensor, 6 ops/pair-chunk)
  5. scores: P[a,b] = QRT[:,a]^T @ QRT[:,b] for a <= b only (fp16, fp32 PSUM).
     By symmetry of S, P[a,b] is exactly the lhsT ([s, t]) needed by the AV matmul.
  6. PSUM -> SBUF fp16 (diagonal blocks masked to strict-upper = s < t)
  7. AV: out[b] = sum_{a<=b} P[a,b].T-form @ V[a], fp32 PSUM accum -> fp32 out
"""

import math
import os
import sys

import numpy as np

for _p in ("/opt/trn_rl_repo",):
    if _p not in sys.path and os.path.isdir(_p):
        sys.path.insert(0, _p)

THETA = 2 ** 16
B, H, T, N, D = 2, 8, 2048, 1024, 128
NB = T // 128          # 16 t-blocks
NC_COUNT = 8
BH_PER_CORE = (B * H) // NC_COUNT  # 2
NPAIR = N // 2         # 512 rotation pairs
NCHUNK = N // 128      # 8 partition chunks of QRT
NPCH = NPAIR // 128    # 4 pair chunks

_cache = {}


def _make_tables():
    """cos/sin tables in transposed, pair-collapsed layout [512, 2048] fp16.

    Phase arithmetic replicates reference._get_freqs/_rope bit-for-bit in fp32
    (jnp ops on CPU), so the only table error is the final fp16 quantization.
    """
    import jax
    import jax.numpy as jnp

    with jax.default_device(jax.devices("cpu")[0]):
        pos = jnp.floor(jnp.arange(N, dtype=jnp.float32) / 2.0) * 2.0
        freqs = 1.0 / (THETA ** (pos / N)) / (2.0 * math.pi)        # (N,) fp32
        r_phases = jnp.arange(T, dtype=jnp.float32)[:, None] * freqs[None, :]
        ph = (r_phases % 1.0) * (2.0 * math.pi)
        c = np.asarray(jnp.cos(ph))                                  # (T, N) fp32
        s = np.asarray(jnp.sin(ph))
    # pair-collapse (cols 2i == 2i+1) and transpose -> [512, 2048]
    ct = np.ascontiguousarray(c[:, 0::2].T).astype(np.float16)
    st = np.ascontiguousarray(s[:, 0::2].T).astype(np.float16)
    return ct, st


def _build_nc():
    import concourse.mybir as mybir
    from concourse import bacc
    from concourse.tile import TileContext

    f32 = mybir.dt.float32
    f16 = mybir.dt.float16

    ct_np, st_np = _make_tables()
    # mask[j, i] = 1 if j < i else 0 (keep strictly-upper: s < t)
    mask_np = np.triu(np.ones((128, 128), np.float16), 1)

    nc = bacc.Bacc("TRN2", target_bir_lowering=False, debug=False,
                   num_devices=NC_COUNT)
    q = nc.dram_tensor("q", [BH_PER_CORE, T, N], f32, kind="ExternalInput")
    v = nc.dram_tensor("v", [BH_PER_CORE, T, D], f32, kind="ExternalInput")
    out = nc.dram_tensor("out", [BH_PER_CORE, T, D], f32, kind="ExternalOutput")
    ct_dram = nc.inline_tensor(ct_np, name="ct_tab")
    st_dram = nc.inline_tensor(st_np, name="st_tab")
    mask_dram = nc.inline_tensor(mask_np, name="mask_tab")

    with TileContext(nc) as tc:
        with tc.tile_pool(name="const", bufs=1) as cpool, \
             tc.tile_pool(name="work", bufs=1) as pool, \
             tc.tile_pool(name="dscr", bufs=2, space="DRAM") as dpool, \
             tc.tile_pool(name="psS", bufs=5, space="PSUM") as psS, \
             tc.tile_pool(name="psO", bufs=2, space="PSUM") as psO:

            # constants
            ct_sb = []
            st_sb = []
            for j in range(NPCH):
                t_c = cpool.tile([128, T], f16, name=f"ct{j}")
                nc.sync.dma_start(out=t_c, in_=ct_dram[j * 128:(j + 1) * 128, :])
                ct_sb.append(t_c)
                t_s = cpool.tile([128, T], f16, name=f"st{j}")
                nc.sync.dma_start(out=t_s, in_=st_dram[j * 128:(j + 1) * 128, :])
                st_sb.append(t_s)
            mask_sb = cpool.tile([128, 128], f16, name="mask")
            nc.sync.dma_start(out=mask_sb, in_=mask_dram[:, :])

            copy_alt = [0]  # round-robin ACT/DVE for PSUM drains

            def drain_copy(dst, src):
                if copy_alt[0] % 2 == 0:
                    nc.scalar.copy(dst, src)
                else:
                    nc.vector.tensor_copy(out=dst, in_=src)
                copy_alt[0] += 1

            for bh in range(BH_PER_CORE):
                # ---- V load with cast (SWDGE dma cast), one call ----
                vf = pool.tile([128, NB * 128], f16, tag="vf", bufs=2,
                               name=f"vf{bh}")
                nc.gpsimd.dma_start(
                    out=vf.rearrange("p (a d) -> p a d", a=NB),
                    in_=v[bh].rearrange("(a p) d -> p a d", p=128),
                )

                # ---- Q load, cast + de-interleave, DRAM bounce, transpose ----
                qrt = []
                for k in range(NCHUNK):
                    qc = pool.tile([128, T], f16, tag="qrt", bufs=2 * NCHUNK,
                                   name=f"qrt{bh}_{k}")
                    qrt.append(qc)
                qde_d = dpool.tile([T, N], f16, tag="qde_d", name=f"qde_d{bh}")
                for tt in range(NB):
                    qf = pool.tile([128, N], f32, tag="qstage", bufs=3,
                                   name=f"qf{bh}_{tt}")
                    nc.sync.dma_start(out=qf, in_=q[bh, tt * 128:(tt + 1) * 128, :])
                    qd = pool.tile([128, N], f16, tag="qde", bufs=3,
                                   name=f"qd{bh}_{tt}")
                    # cast + de-interleave on ACT: evens -> [0:512], odds -> rest
                    nc.scalar.copy(qd[:, 0:NPAIR], qf[:, 0::2])
                    nc.scalar.copy(qd[:, NPAIR:N], qf[:, 1::2])
                    # store on the ACT HWDGE ring: same engine as the casts,
                    # so no cross-engine wait blocks the ring
                    nc.scalar.dma_start(out=qde_d[tt * 128:(tt + 1) * 128, :],
                                        in_=qd)
                for k in range(NCHUNK):
                    nc.sync.dma_start_transpose(
                        qrt[k], qde_d[0:T, k * 128:(k + 1) * 128])

                # ---- rope in transposed layout ----
                for j in range(NPCH):
                    qe, qo = qrt[j], qrt[j + NPCH]
                    c_t, s_t = ct_sb[j], st_sb[j]
                    t1 = pool.tile([128, T], f16, tag="tmp1", bufs=2,
                                   name=f"t1_{bh}_{j}")
                    t2 = pool.tile([128, T], f16, tag="tmp2", bufs=2,
                                   name=f"t2_{bh}_{j}")
                    nc.vector.tensor_mul(out=t1, in0=qe, in1=s_t)
                    nc.vector.tensor_mul(out=t2, in0=qo, in1=s_t)
                    nc.vector.tensor_mul(out=qe, in0=qe, in1=c_t)
                    nc.vector.tensor_sub(out=qe, in0=qe, in1=t2)
                    nc.vector.tensor_mul(out=qo, in0=qo, in1=c_t)
                    nc.vector.tensor_add(out=qo, in0=qo, in1=t1)

                # ---- scores: P[a, b] for a <= b ----
                strips = []
                for a in range(NB):
                    strip = pool.tile([128, (NB - a) * 128], f16,
                                      tag=f"strip{a}", bufs=1,
                                      name=f"strip{bh}_{a}")
                    strips.append(strip)
                    asl = slice(a * 128, (a + 1) * 128)
                    groups = []
                    for gs in range(a, NB, 4):
                        w = min(4, NB - gs) * 128
                        ps = psS.tile([128, 512], f32, tag="ps",
                                      name=f"ps{bh}_{a}_{gs}")
                        groups.append((gs, w, ps))
                    korder = [0, 4, 1, 5, 2, 6, 3, 7]  # pair j readies chunks j, j+4
                    for ki, k in enumerate(korder):
                        for (gs, w, ps) in groups:
                            nc.tensor.matmul(
                                ps[:, :w],
                                lhsT=qrt[k][:, asl],
                                rhs=qrt[k][:, gs * 128: gs * 128 + w],
                                start=(ki == 0),
                                stop=(ki == NCHUNK - 1),
                            )
                    for (gs, w, ps) in groups:
                        off = (gs - a) * 128      # strip column offset
                        if gs == a:
                            # diagonal block: strict-upper mask (s < t)
                            nc.vector.tensor_mul(
                                out=strip[:, 0:128], in0=ps[:, 0:128],
                                in1=mask_sb,
                            )
                            if w > 128:
                                drain_copy(strip[:, 128:w], ps[:, 128:w])
                        else:
                            drain_copy(strip[:, off:off + w], ps[:, :w])

                # ---- AV: out[b] = sum_{a<=b} P[a,b]-as-lhsT @ V[a] ----
                for b in range(NB):
                    po = psO.tile([128, D], f32, tag="po", name=f"po{bh}_{b}")
                    for a in range(b + 1):
                        nc.tensor.matmul(
                            po,
                            lhsT=strips[a][:, (b - a) * 128:(b - a + 1) * 128],
                            rhs=vf[:, a * 128:(a + 1) * 128],
                            start=(a == 0),
                            stop=(a == b),
                        )
                    ob = pool.tile([128, D], f32, tag="ostage", bufs=4,
                                   name=f"ob{bh}_{b}")
                    drain_copy(ob, po)
                    nc.sync.dma_start(out=out[bh, b * 128:(b + 1) * 128, :], in_=ob)

    nc.compile()
    return nc


def _get_nc():
    if "nc" not in _cache:
        _cache["nc"] = _build_nc()
    return _cache["nc"]


def kernel(Q, K, V):
    from concourse import bass_utils

    del K  # K is Q by construction
    Qr = np.ascontiguousarray(Q.reshape(B * H, T, N), dtype=np.float32)
    Vr = np.ascontiguousarray(V.reshape(B * H, T, D), dtype=np.float32)

    nc = _get_nc()
    in_maps = []
    for c in range(NC_COUNT):
        lo = c * BH_PER_CORE
        in_maps.append({
            "q": np.ascontiguousarray(Qr[lo:lo + BH_PER_CORE]),
            "v": np.ascontiguousarray(Vr[lo:lo + BH_PER_CORE]),
        })

    res = bass_utils.run_bass_kernel_spmd(
        nc, in_maps, core_ids=list(range(NC_COUNT)),
    )
    _cache["last_result"] = res
    outs = [res.results[c]["out"].reshape(BH_PER_CORE, T, D)
            for c in range(NC_COUNT)]
    return np.concatenate(outs, axis=0).reshape(B, H, T, D).astype(np.float32)


# revision 27
# speedup vs baseline: 1.4401x; 1.4401x over previous
"""Trainium2 Bass kernel for RoPE linear attention (no softmax, strict causal).

Computes: QR = rope(Q); S = tril(QR @ QR^T, -1); out = S @ V
for Q [B=2, H=8, T=2048, N=1024], V [B,H,T,D=128], K == Q.

Sharding: B*H = 16 (b,h) pairs -> 2 per core across 8 cores (fully parallel).

Per-core device pipeline (per (b,h)):
  1. load Q tiles fp32 [128, 1024]
  2. cast fp32->fp16 with pair de-interleave (evens -> cols 0:512, odds -> rest)
  3. PE transpose (fp16, grouped 4 chunks/psum bank) -> QRT [n', t]
     as one tensor qrt_big[:, k*T:(k+1)*T] = chunk k
  4. rope in transposed layout (unit-stride fp16 tensor_tensor, in place)
  5. scores: P[a,b] = QRT[:,a]^T @ QRT[:,b] for a <= b only (fp16, fp32 PSUM).
     By symmetry of S, P[a,b] is exactly the lhsT ([s, t]) the AV matmul needs.
  6. PSUM -> SBUF fp16 (diagonal blocks masked to strict-upper = s < t)
  7. AV: out[b] = sum_{a<=b} P[a,b] @ V[a], fp32 PSUM accum -> fp32 out

Emission order A0, S0, A1, AV0, S1, AV1 keeps every in-order engine/ring
free of cross-phase head-of-line blocking.
"""

import math
import os
import sys

import numpy as np

for _p in ("/opt/trn_rl_repo",):
    if _p not in sys.path and os.path.isdir(_p):
        sys.path.insert(0, _p)

THETA = 2 ** 16
B, H, T, N, D = 2, 8, 2048, 1024, 128
NB = T // 128          # 16 t-blocks
NC_COUNT = 8
BH_PER_CORE = (B * H) // NC_COUNT  # 2
NPAIR = N // 2         # 512 rotation pairs
NCHUNK = N // 128      # 8 partition chunks of QRT
NPCH = NPAIR // 128    # 4 pair chunks

_cache = {}


def _make_tables():
    """cos/sin tables in transposed, pair-collapsed layout [512, 2048] fp16.

    Phase arithmetic replicates reference._get_freqs/_rope bit-for-bit in fp32
    (jnp ops on CPU), so the only table error is the final fp16 quantization.
    """
    import jax
    import jax.numpy as jnp

    with jax.default_device(jax.devices("cpu")[0]):
        pos = jnp.floor(jnp.arange(N, dtype=jnp.float32) / 2.0) * 2.0
        freqs = 1.0 / (THETA ** (pos / N)) / (2.0 * math.pi)        # (N,) fp32
        r_phases = jnp.arange(T, dtype=jnp.float32)[:, None] * freqs[None, :]
        ph = (r_phases % 1.0) * (2.0 * math.pi)
        c = np.asarray(jnp.cos(ph))                                  # (T, N) fp32
        s = np.asarray(jnp.sin(ph))
    ct = np.ascontiguousarray(c[:, 0::2].T).astype(np.float16)
    st = np.ascontiguousarray(s[:, 0::2].T).astype(np.float16)
    return ct, st


def _build_nc():
    import concourse.mybir as mybir
    from concourse import bacc
    from concourse.tile import TileContext

    f32 = mybir.dt.float32
    f16 = mybir.dt.float16

    ct_np, st_np = _make_tables()
    # mask[j, i] = 1 if j < i else 0 (keep strictly-upper: s < t)
    mask_np = np.triu(np.ones((128, 128), np.float16), 1)
    ident_np = np.eye(128, dtype=np.float16)

    nc = bacc.Bacc("TRN2", target_bir_lowering=False, debug=False,
                   num_devices=NC_COUNT)
    q = nc.dram_tensor("q", [BH_PER_CORE, T, N], f32, kind="ExternalInput")
    v = nc.dram_tensor("v", [BH_PER_CORE, T, D], f32, kind="ExternalInput")
    out = nc.dram_tensor("out", [BH_PER_CORE, T, D], f32, kind="ExternalOutput")
    ct_dram = nc.inline_tensor(ct_np, name="ct_tab")
    st_dram = nc.inline_tensor(st_np, name="st_tab")
    mask_dram = nc.inline_tensor(mask_np, name="mask_tab")
    ident_dram = nc.inline_tensor(ident_np, name="ident_tab")

    with TileContext(nc) as tc:
        with tc.tile_pool(name="const", bufs=1) as cpool, \
             tc.tile_pool(name="work", bufs=1) as pool, \
             tc.tile_pool(name="psS", bufs=4, space="PSUM") as psS, \
             tc.tile_pool(name="psT", bufs=2, space="PSUM") as psT, \
             tc.tile_pool(name="psO", bufs=2, space="PSUM") as psO:

            # constants
            ct_sb = []
            st_sb = []
            for j in range(NPCH):
                t_c = cpool.tile([128, T], f16, name=f"ct{j}")
                nc.sync.dma_start(out=t_c, in_=ct_dram[j * 128:(j + 1) * 128, :])
                ct_sb.append(t_c)
                t_s = cpool.tile([128, T], f16, name=f"st{j}")
                nc.sync.dma_start(out=t_s, in_=st_dram[j * 128:(j + 1) * 128, :])
                st_sb.append(t_s)
            mask_sb = cpool.tile([128, 128], f16, name="mask")
            nc.sync.dma_start(out=mask_sb, in_=mask_dram[:, :])
            ident_sb = cpool.tile([128, 128], f16, name="ident")
            nc.sync.dma_start(out=ident_sb, in_=ident_dram[:, :])

            copy_alt = [0]  # round-robin ACT/DVE for PSUM drains

            def drain_copy(dst, src):
                if copy_alt[0] % 2 == 0:
                    nc.scalar.copy(dst, src)
                else:
                    nc.vector.tensor_copy(out=dst, in_=src)
                copy_alt[0] += 1

            TH = T // 2

            def load_cast(bh, split_cast):
                """V load + Q load + cast. Returns (vf, qrt views, qd list)."""
                vf = pool.tile([128, NB * 128], f16, tag="vf", bufs=2,
                               name=f"vf{bh}")
                nc.gpsimd.dma_start(
                    out=vf.rearrange("p (a d) -> p a d", a=NB),
                    in_=v[bh].rearrange("(a p) d -> p a d", p=128),
                )
                qrt_big = pool.tile([128, NCHUNK * T], f16, tag="qrt", bufs=2,
                                    name=f"qrtbig{bh}")
                qrt = [qrt_big[:, k * T:(k + 1) * T] for k in range(NCHUNK)]
                qrt_3d = qrt_big.rearrange("p (c t) -> p c t", c=NCHUNK)
                qds = []
                for tt in range(NB):
                    qf = pool.tile([128, N], f32, tag="qstage", bufs=3,
                                   name=f"qf{bh}_{tt}")
                    nc.sync.dma_start(out=qf,
                                      in_=q[bh, tt * 128:(tt + 1) * 128, :])
                    qd = pool.tile([128, N], f16, tag="qde", bufs=8,
                                   name=f"qd{bh}_{tt}")
                    # cast + de-interleave: evens->[0:512], odds->rest
                    nc.scalar.copy(qd[:, 0:NPAIR], qf[:, 0::2])
                    nc.scalar.copy(qd[:, NPAIR:N], qf[:, 1::2])
                    qds.append(qd)
                return vf, qrt, qrt_3d, qds

            def emit_transpose(bh, qrt_3d, qds, tt):
                """PE transposes of tile tt, 4 chunks per PSUM bank."""
                qd = qds[tt]
                for g in range(2):
                    pt = psT.tile([128, 512], f16, tag="pt",
                                  name=f"pt{bh}_{tt}_{g}")
                    for kk in range(4):
                        k = g * 4 + kk
                        nc.tensor.transpose(
                            pt[:, kk * 128:(kk + 1) * 128],
                            qd[:, k * 128:(k + 1) * 128],
                            ident_sb)
                    drain_copy(
                        qrt_3d[:, g * 4:(g + 1) * 4,
                               tt * 128:(tt + 1) * 128],
                        pt.rearrange("p (c t) -> p c t", c=4))

            def emit_rope(bh, qrt, h):
                hsl = slice(h * TH, (h + 1) * TH)
                for j in range(NPCH):
                    qe, qo = qrt[j][:, hsl], qrt[j + NPCH][:, hsl]
                    c_t, s_t = ct_sb[j][:, hsl], st_sb[j][:, hsl]
                    t1 = pool.tile([128, TH], f16, tag="tmp1", bufs=2,
                                   name=f"t1_{bh}_{j}_{h}")
                    t2 = pool.tile([128, TH], f16, tag="tmp2", bufs=2,
                                   name=f"t2_{bh}_{j}_{h}")
                    nc.vector.tensor_mul(out=t1, in0=qe, in1=s_t)
                    nc.vector.tensor_mul(out=t2, in0=qo, in1=s_t)
                    nc.vector.tensor_mul(out=qe, in0=qe, in1=c_t)
                    nc.vector.tensor_sub(out=qe, in0=qe, in1=t2)
                    nc.vector.tensor_mul(out=qo, in0=qo, in1=c_t)
                    nc.vector.tensor_add(out=qo, in0=qo, in1=t1)

            def phase_s(bh, qrt, inject=None):
                """Scores P[a,b] for a <= b, in two passes by b-half.

                Pass 0 covers b in [a..7], pass 1 covers b in [max(a,8)..15].
                With korder [0,4,1,5,...], each rope step (pair j, half h)
                readies chunks j and j+4, so pass-0 matmuls unlock while
                rope is still running.

                inject(hpass, a): emits other-bh pipeline work between
                a-iterations (keeps the in-order PE stream fed).
                """
                strips = [
                    pool.tile([128, (NB - a) * 128], f16, tag=f"strip{a}",
                              bufs=1, name=f"strip{bh}_{a}")
                    for a in range(NB)
                ]
                korder = [0, 4, 1, 5, 2, 6, 3, 7]
                HB = NB // 2
                for hpass in range(2):
                    for a in range(NB):
                        if inject is not None:
                            inject(hpass, a)
                        blo = a if hpass == 0 else max(a, HB)
                        bhi = HB if hpass == 0 else NB
                        if blo >= bhi:
                            continue
                        strip = strips[a]
                        asl = slice(a * 128, (a + 1) * 128)
                        groups = []
                        for gs in range(blo, bhi, 4):
                            w = min(4, bhi - gs) * 128
                            ps = psS.tile([128, 512], f32, tag="ps",
                                          name=f"ps{bh}_{hpass}_{a}_{gs}")
                            groups.append((gs, w, ps))
                        for ki, k in enumerate(korder):
                            for (gs, w, ps) in groups:
                                nc.tensor.matmul(
                                    ps[:, :w],
                                    lhsT=qrt[k][:, asl],
                                    rhs=qrt[k][:, gs * 128: gs * 128 + w],
                                    start=(ki == 0),
                                    stop=(ki == NCHUNK - 1),
                                )
                        for (gs, w, ps) in groups:
                            off = (gs - a) * 128      # strip column offset
                            if gs == a:
                                # diagonal block: strict-upper mask (s < t)
                                nc.vector.tensor_mul(
                                    out=strip[:, off:off + 128],
                                    in0=ps[:, 0:128], in1=mask_sb,
                                )
                                if w > 128:
                                    drain_copy(strip[:, off + 128:off + w],
                                               ps[:, 128:w])
                            else:
                                drain_copy(strip[:, off:off + w], ps[:, :w])
                return strips

            def phase_av(bh, vf, strips):
                for b in range(NB):
                    po = psO.tile([128, D], f32, tag="po", name=f"po{bh}_{b}")
                    for a in range(b + 1):
                        nc.tensor.matmul(
                            po,
                            lhsT=strips[a][:, (b - a) * 128:(b - a + 1) * 128],
                            rhs=vf[:, a * 128:(a + 1) * 128],
                            start=(a == 0),
                            stop=(a == b),
                        )
                    ob = pool.tile([128, D], f32, tag="ostage", bufs=4,
                                   name=f"ob{bh}_{b}")
                    drain_copy(ob, po)
                    nc.sync.dma_start(out=out[bh, b * 128:(b + 1) * 128, :],
                                      in_=ob)

            # bh0: self-paced pipeline (PE idle during fill anyway)
            vf0, qrt0, qrt0_3d, qds0 = load_cast(0, split_cast=False)
            for tt in range(NB):
                emit_transpose(0, qrt0_3d, qds0, tt)
            for h in range(2):
                emit_rope(0, qrt0, h)

            # bh1: loads+casts up front; transposes injected into the
            # scores(bh0) PE stream at the loads' pace; rope per half once
            # its 8 tiles are transposed
            vf1, qrt1, qrt1_3d, qds1 = load_cast(1, split_cast=False)

            def inject(hpass, a):
                # late enough that bh1's loads/casts are done; transposes
                # ride the S0 PE stream, rope overlaps AV0
                if hpass == 1 and 4 <= a <= 11:
                    for tt in (2 * (a - 4), 2 * (a - 4) + 1):
                        emit_transpose(1, qrt1_3d, qds1, tt)
                    if a == 7:
                        emit_rope(1, qrt1, 0)
                    elif a == 11:
                        emit_rope(1, qrt1, 1)

            strips0 = phase_s(0, qrt0, inject=inject)
            phase_av(0, vf0, strips0)
            strips1 = phase_s(1, qrt1)
            phase_av(1, vf1, strips1)

    nc.compile()
    return nc


def _get_nc():
    if "nc" not in _cache:
        _cache["nc"] = _build_nc()
    return _cache["nc"]


def kernel(Q, K, V):
    from concourse import bass_utils

    del K  # K is Q by construction
    Qr = np.ascontiguousarray(Q.reshape(B * H, T, N), dtype=np.float32)
    Vr = np.ascontiguousarray(V.reshape(B * H, T, D), dtype=np.float32)

    nc = _get_nc()
    in_maps = []
    for c in range(NC_COUNT):
        lo = c * BH_PER_CORE
        in_maps.append({
            "q": np.ascontiguousarray(Qr[lo:lo + BH_PER_CORE]),
            "v": np.ascontiguousarray(Vr[lo:lo + BH_PER_CORE]),
        })

    res = bass_utils.run_bass_kernel_spmd(
        nc, in_maps, core_ids=list(range(NC_COUNT)),
    )
    _cache["last_result"] = res
    outs = [res.results[c]["out"].reshape(BH_PER_CORE, T, D)
            for c in range(NC_COUNT)]
    return np.concatenate(outs, axis=0).reshape(B, H, T, D).astype(np.float32)
